# revision 8
# baseline (speedup 1.0000x reference)
"""Trainium2 Bass kernel for nn_Loss_Synonymy.

reference:
    diff = S1 - S2                       # [B, 256]
    d    = sqrt(sum(diff^2, axis=-1))    # [B]
    t    = tanh(d)
    err  = where(score >= 0.8, relu(1 - t), relu(1 + t))
    out  = sum(err) / B

Since tanh(d) in [0, 1) for d >= 0, relu(1 -+ tanh(d)) = 1 -+ tanh(d), so
err = 1 + sgn * tanh(d) with sgn = -1 (score >= 0.8) else +1, and
sum(err) = B + sum(sgn * tanh(d)).  The kernel only accumulates
sgn * tanh(d); the host adds B and divides.

Data-parallel over 8 NeuronCores, 32768 rows each.  Per-core layout:
partition p owns rows [p*256, (p+1)*256) of the shard, so the score
vector is ONE contiguous [128, 256] HWDGE load and the per-row sums
land as [128, 256] aligned with it.  s1/s2 are stacked host-side into
x[2, BL, D] so each tile is a single 4 MiB dma_start (2 x 16 KiB
contiguous per partition).  No SWDGE/gpsimd DMAs at all -- they were
the cause of a persistent ~20% straggler on SDMA engine 15.

Tile t covers row-offsets [off, off+J) of every partition's 256-row
block.  Per big tile (J=16):

    DMA  (sync HWDGE): X[128, 2*J*256] f32 <- x[:, p*256+off .. +J, :]
    DVE : diff = a - b   -> bf16 tile (X released right after)
    ACT : sq   = Square(diff)   in place, bf16
    DVE : sumsq[:, off:off+J] = reduce_add(sq.view(128, J, 256), axis=X)
          (bf16 input = 2x DVE read rate; f32 accumulate/output)

The reduce for tile t is emitted after sub/square of tile t+1
(software pipelining) so the in-order DVE never waits on ACT.
4 taper tiles (J=4) at the end shrink the drain; their squares run on
DVE so ACT's table-set switches (Square -> rsqrt -> Tanh, ~1.3 us
each) are hoisted into dummy activations that overlap the taper.

Epilogue: d = sumsq * rsqrt(sumsq + 1e-12) (avoids the Sqrt table set;
tanh saturates at d ~ 16 so rsqrt accuracy is irrelevant; the bias
keeps sumsq == 0 finite), th = Tanh(d), then
(score >= 0.8 ? -1 : +1) * th accumulated per partition -> [128, 1].
Host: out = (B + sum(partials)) / B.
"""

import numpy as np

import concourse.bass as bass
import concourse.tile as tile
from concourse import bacc, mybir
from concourse.bass_utils import run_bass_kernel_spmd

F32 = mybir.dt.float32
BF16 = mybir.dt.bfloat16
AF = mybir.ActivationFunctionType
ALU = mybir.AluOpType

B = 262144
D = 256
NCORES = 8
BL = B // NCORES          # 32768 rows per core
RPP = BL // 128           # 256 rows per partition
THRESH = 0.8

# (J, count): per-partition row-chunks per tile; sum(J*count) == RPP
TILING = [(16, 15), (4, 4)]
BIG_J = TILING[0][0]
BUFS_X = 4
BUFS_DIFF = 3
BUFS_XS = 2

_NC_CACHE = {}


def _build_nc():
    nc = bacc.Bacc(
        "TRN2", target_bir_lowering=False, debug=False, num_devices=NCORES
    )

    x = nc.dram_tensor("x", [2, BL, D], F32, kind="ExternalInput").ap()
    score = nc.dram_tensor("score", [BL], F32, kind="ExternalInput").ap()
    partial = nc.dram_tensor("partial", [128, 1], F32, kind="ExternalOutput").ap()

    # [128, 2, 256, 256]: partition p / source s / row-in-block c / feature d
    x_r = x.rearrange("s (p c) d -> p s c d", p=128, c=RPP)
    score_r = score.rearrange("(p c) -> p c", p=128, c=RPP)

    with tile.TileContext(nc) as tc:
        with (
            tc.tile_pool(name="xin", bufs=BUFS_X) as p_x,
            tc.tile_pool(name="xsmall", bufs=BUFS_XS) as p_xs,
            tc.tile_pool(name="diff", bufs=BUFS_DIFF) as p_diff,
            tc.tile_pool(name="dsmall", bufs=BUFS_XS) as p_ds,
            tc.tile_pool(name="persist", bufs=1) as p_per,
        ):
            sumsq = p_per.tile([128, RPP], F32, tag="sumsq")
            score_sb = p_per.tile([128, RPP], F32, tag="score_sb")
            part_sb = p_per.tile([128, 1], F32, tag="part_sb")

            # Contiguous score load on the HWDGE ring (1 KiB per partition).
            nc.sync.dma_start(score_sb[:], score_r)

            # (score >= 0.8) * -2  ->  {-2, 0}; hidden under the DMA ramp.
            sgn2 = p_per.tile([128, RPP], F32, tag="sgn2")
            nc.vector.tensor_scalar(
                sgn2[:], score_sb[:], THRESH, -2.0, ALU.is_ge, ALU.mult
            )

            pending = None  # (diff_bf16, off, J) awaiting its reduce

            def emit_reduce(p):
                dt, off, J = p
                nc.vector.tensor_reduce(
                    sumsq[:, off : off + J],
                    dt[:].rearrange("p (j d) -> p j d", d=D),
                    axis=mybir.AxisListType.X,
                    op=ALU.add,
                )

            off = 0
            for J, count in TILING:
                FREE = J * D
                big = J == BIG_J
                for _ in range(count):
                    X = (p_x if big else p_xs).tile(
                        [128, 2 * FREE], F32, tag=f"x{J}"
                    )
                    nc.sync.dma_start(
                        X[:].rearrange("p (s j d) -> p s j d", s=2, d=D),
                        x_r[:, :, off : off + J, :],
                    )
                    dt = (p_diff if big else p_ds).tile(
                        [128, FREE], BF16, tag=f"d{J}"
                    )
                    nc.vector.tensor_sub(dt[:], X[:, 0:FREE], X[:, FREE:])
                    if big:
                        nc.scalar.activation(dt[:], dt[:], AF.Square)
                    else:
                        # taper squares on DVE: frees ACT to prefetch the
                        # rsqrt/Tanh table sets below
                        nc.vector.scalar_tensor_tensor(
                            dt[:], dt[:], 1.0, dt[:], ALU.mult, ALU.mult
                        )
                    if pending is not None:
                        emit_reduce(pending)
                    pending = (dt, off, J)
                    off += J
            emit_reduce(pending)

            # Epilogue: part = sum_p sgn * tanh(d), d = sumsq * rsqrt(sumsq).
            rs = p_per.tile([128, RPP], F32, tag="rs")
            nc.scalar.activation(rs[:], sumsq[:], AF.Abs_reciprocal_sqrt)
            # min(rs, 1e6) clamps rsqrt(0)=inf so sumsq==0 -> dist=0 -> tanh=0,
            # exactly matching the reference for degenerate rows.
            dist = p_per.tile([128, RPP], F32, tag="dist")
            nc.vector.scalar_tensor_tensor(
                dist[:], rs[:], 1e6, sumsq[:], ALU.min, ALU.mult
            )
            th = p_per.tile([128, RPP], F32, tag="th")
            nc.scalar.activation(th[:], dist[:], AF.Tanh)
            # (sgn2 + 1) * th -> +-tanh, accumulated per partition
            err = p_per.tile([128, RPP], F32, tag="err")
            nc.vector.scalar_tensor_tensor(
                err[:], sgn2[:], 1.0, th[:], ALU.add, ALU.mult,
                accum_out=part_sb[:],
            )

            nc.sync.dma_start(partial, part_sb[:])

    nc.compile()
    return nc


def _get_nc():
    if "nc" not in _NC_CACHE:
        _NC_CACHE["nc"] = _build_nc()
    return _NC_CACHE["nc"]


def make_in_maps(S1_out, S2_out, synonymy_score):
    in_maps = []
    for c in range(NCORES):
        lo, hi = c * BL, (c + 1) * BL
        x = np.empty((2, BL, D), dtype=np.float32)
        x[0] = S1_out[lo:hi]
        x[1] = S2_out[lo:hi]
        in_maps.append(
            {
                "x": x,
                "score": np.ascontiguousarray(
                    synonymy_score[lo:hi], dtype=np.float32
                ),
            }
        )
    return in_maps


def combine(results):
    total = np.float64(B)
    for r in results:
        total += r["partial"].astype(np.float64).sum()
    return np.asarray(total / B, dtype=np.float32)


def run(S1_out, S2_out, synonymy_score, trace=False, **trace_kwargs):
    nc = _get_nc()
    in_maps = make_in_maps(S1_out, S2_out, synonymy_score)
    res = run_bass_kernel_spmd(
        nc, in_maps, list(range(NCORES)), trace=trace, **trace_kwargs
    )
    return combine(res.results), res


def kernel(S1_out, S2_out, synonymy_score):
    out, _ = run(S1_out, S2_out, synonymy_score)
    return out
